# revision 1
# baseline (speedup 1.0000x reference)
"""BoundaryLoss kernel for 8 Trainium2 NeuronCores: hybrid gather.

Computes mean_i relu(MARGIN - inputs[i, labels[i]]) over [65536, 1024] f32
inputs, data parallel across 8 cores (8192 rows per core).

The two working gather strategies bottleneck on different engines:
  - indirect_dma_start (128 elements/instruction): Pool-engine serial,
    ~1.2-1.4 us per instruction, negligible HBM traffic.
  - full-read + fused select (scalar_tensor_tensor): DMA-engine bound,
    ~1.4 us per [128, 1024] tile.
Split the 64 row-tiles between the two paths so both run concurrently.
"""

import os
import sys

for _p in ("/opt/trn_rl_repo", os.path.expanduser("~/.axon_site/_ro/trn_rl_repo")):
    if os.path.isdir(_p) and _p not in sys.path:
        sys.path.insert(0, _p)

import numpy as np

import concourse.bacc as bacc
import concourse.bass as bass
import concourse.mybir as mybir
import concourse.tile as tile
from concourse import bass_utils

POSITIVE_MARGIN = 0.99999
N, G = 65536, 1024
NCORES = 8
NS = N // NCORES
P = 128
T = NS // P  # 64 row-tiles total

# tiles handled by the full-read path; the rest go to the indirect path
FULL_TILES = 30
IND_TILES = T - FULL_TILES  # indirect path: rows FULL_TILES*128 .. NS


def build_program(full_tiles: int = FULL_TILES):
    f32 = mybir.dt.float32
    i32 = mybir.dt.int32
    ind_tiles = T - full_tiles
    ind_base = full_tiles * P  # first row of the indirect block

    nc = bacc.Bacc(
        "TRN2",
        target_bir_lowering=False,
        debug=False,
        dynamic_dma_scratch_size=32768,
    )
    x_t = nc.dram_tensor("inputs", [NS, G], f32, kind="ExternalInput")
    lab_t = nc.dram_tensor("labels_lo_hi", [NS, 2], i32, kind="ExternalInput")
    out_t = nc.dram_tensor("partials", [P, 1], f32, kind="ExternalOutput")

    with tile.TileContext(nc) as tc:
        with tc.tile_pool(name="pool", bufs=1) as pool, tc.tile_pool(
            name="xbuf", bufs=6
        ) as xbuf:
            # vals[:, 0:full_tiles] <- full-read path
            # vals[:, full_tiles:T] <- indirect path
            vals = pool.tile([P, T], f32)

            # ---------- indirect path ----------
            if ind_tiles:
                # rows ind_base + p*ind_tiles + j  (p = partition, j = col)
                lab_ind = pool.tile([P, 2 * ind_tiles], i32)
                nc.sync.dma_start(
                    out=lab_ind[:].rearrange("p (f t) -> p f t", t=2),
                    in_=lab_t.ap()[ind_base:, :].rearrange(
                        "(p f) t -> p f t", p=P
                    ),
                )
                offs = pool.tile([P, ind_tiles], i32)
                nc.gpsimd.iota(
                    offs[:],
                    pattern=[[G, ind_tiles]],
                    base=ind_base * G,
                    channel_multiplier=ind_tiles * G,
                )
                nc.vector.tensor_tensor(
                    out=offs[:],
                    in0=offs[:],
                    in1=lab_ind[:].rearrange("p (f t) -> p f t", t=2)[:, :, 0],
                    op=mybir.AluOpType.add,
                )
                for j in range(ind_tiles):
                    nc.gpsimd.indirect_dma_start(
                        out=vals[:, full_tiles + j : full_tiles + j + 1],
                        out_offset=None,
                        in_=x_t.ap(),
                        in_offset=bass.IndirectOffsetOnAxis(
                            ap=offs[:, j : j + 1], axis=1
                        ),
                    )

            # ---------- full-read path ----------
            if full_tiles:
                # lab_f[p, t] = labels[t*128 + p] as f32
                lab_raw = pool.tile([P, full_tiles * 2], i32)
                nc.sync.dma_start(
                    out=lab_raw[:].rearrange("p (t c) -> p t c", c=2),
                    in_=lab_t.ap()[: full_tiles * P, :].rearrange(
                        "(t p) c -> p t c", p=P
                    ),
                )
                lab_f = pool.tile([P, full_tiles], f32)
                nc.vector.tensor_copy(
                    out=lab_f[:],
                    in_=lab_raw[:].rearrange("p (t c) -> p t c", c=2)[:, :, 0],
                )

                iota_i = pool.tile([P, G], i32)
                nc.gpsimd.iota(
                    iota_i[:], pattern=[[1, G]], base=0, channel_multiplier=0
                )
                iota_f = pool.tile([P, G], f32)
                nc.vector.tensor_copy(out=iota_f[:], in_=iota_i[:])

                for t in range(full_tiles):
                    xt = xbuf.tile([P, G], f32, tag="xt")
                    nc.sync.dma_start(
                        out=xt[:], in_=x_t.ap()[t * P : (t + 1) * P, :]
                    )
                    dummy = xbuf.tile([P, G], f32, tag="dummy")
                    nc.vector.scalar_tensor_tensor(
                        out=dummy[:],
                        in0=iota_f[:],
                        scalar=lab_f[:, t : t + 1],
                        in1=xt[:],
                        op0=mybir.AluOpType.is_equal,
                        op1=mybir.AluOpType.mult,
                        accum_out=vals[:, t : t + 1],
                    )

            # ---------- combine ----------
            clamp_t = pool.tile([P, T], f32)
            nc.vector.tensor_scalar(
                out=clamp_t[:],
                in0=vals[:],
                scalar1=POSITIVE_MARGIN,
                scalar2=0.0,
                op0=mybir.AluOpType.subtract,
                op1=mybir.AluOpType.min,
            )
            acc = pool.tile([P, 1], f32)
            nc.vector.reduce_sum(acc[:], clamp_t[:], axis=mybir.AxisListType.X)
            # ACT's HWDGE ring is empty: the tiny out-DMA skips the x-tile
            # backlog sitting in sync's FIFO ring
            nc.scalar.dma_start(out=out_t.ap(), in_=acc[:])

    nc.compile()
    return nc


_PROG = None


def _get_prog():
    global _PROG
    if _PROG is None:
        _PROG = build_program()
    return _PROG


def _make_in_maps(inputs: np.ndarray, labels: np.ndarray):
    inputs = np.asarray(inputs)
    labels = np.asarray(labels)
    assert inputs.shape == (N, G), inputs.shape
    assert labels.shape == (N,), labels.shape
    inputs = np.ascontiguousarray(inputs, dtype=np.float32)

    if labels.dtype == np.int64:
        lab2 = np.ascontiguousarray(labels).view(np.int32).reshape(N, 2)
    else:
        lab2 = np.zeros((N, 2), dtype=np.int32)
        lab2[:, 0] = labels.astype(np.int32)
    lab2 = np.ascontiguousarray(lab2)

    in_maps = []
    for c in range(NCORES):
        sl = slice(c * NS, (c + 1) * NS)
        in_maps.append({"inputs": inputs[sl], "labels_lo_hi": lab2[sl]})
    return in_maps


def _run(inputs, labels, trace: bool = False):
    nc = _get_prog()
    in_maps = _make_in_maps(inputs, labels)
    res = bass_utils.run_bass_kernel_spmd(
        nc, in_maps, core_ids=list(range(NCORES)), trace=trace
    )
    total = 0.0
    for r in res.results:
        total += float(np.asarray(r["partials"], dtype=np.float64).sum())
    out = np.array(-total / N, dtype=np.float32)
    return out, res


def kernel(inputs, labels):
    out, _ = _run(inputs, labels, trace=False)
    return out



# revision 2
# speedup vs baseline: 2.9980x; 2.9980x over previous
"""BoundaryLoss kernel for 8 Trainium2 NeuronCores: batched indirect gather.

Computes mean_i relu(MARGIN - inputs[i, labels[i]]) over [65536, 1024] f32
inputs, data parallel across 8 cores (8192 rows per core).

Only 8192 f32 elements per core are actually needed, so the kernel is
gather-instruction-bound, not bandwidth-bound.  Each gpsimd
indirect_dma_start pays ~1us fixed SWDGE descriptor-generation overhead
plus ~0.34ns/descriptor, so all 8192 offsets go into a handful of large
instructions instead of 64 x 128-offset ones.
"""

import os
import sys

for _p in ("/opt/trn_rl_repo", os.path.expanduser("~/.axon_site/_ro/trn_rl_repo")):
    if os.path.isdir(_p) and _p not in sys.path:
        sys.path.insert(0, _p)

import numpy as np

import concourse.bacc as bacc
import concourse.bass as bass
import concourse.mybir as mybir
import concourse.tile as tile
from concourse import bass_utils

POSITIVE_MARGIN = 0.99999
N, G = 65536, 1024
NCORES = 8
NS = N // NCORES
P = 128
F = NS // P  # 64 gathered values per partition

GATHER_CHUNKS = int(os.environ.get("GATHER_CHUNKS", "2"))


def build_program(gather_chunks: int = GATHER_CHUNKS):
    f32 = mybir.dt.float32
    i32 = mybir.dt.int32
    assert F % gather_chunks == 0
    fc = F // gather_chunks  # offset columns per gather instruction

    nc = bacc.Bacc(
        "TRN2",
        target_bir_lowering=False,
        debug=False,
        dynamic_dma_scratch_size=32768,
    )
    x_t = nc.dram_tensor("inputs", [NS, G], f32, kind="ExternalInput")
    lab_t = nc.dram_tensor("labels_lo_hi", [NS, 2], i32, kind="ExternalInput")
    out_t = nc.dram_tensor("partials", [P, 1], f32, kind="ExternalOutput")

    with tile.TileContext(nc) as tc:
        with tc.tile_pool(name="pool", bufs=1) as pool:
            # row r = p*F + f lives at slot (p, f); offs[p, f] = r*G + label[r]
            lab_raw = pool.tile([P, 2 * F], i32)
            nc.sync.dma_start(
                out=lab_raw[:].rearrange("p (f c) -> p f c", c=2),
                in_=lab_t.ap().rearrange("(p f) c -> p f c", p=P),
            )
            offs = pool.tile([P, F], i32)
            nc.gpsimd.iota(
                offs[:],
                pattern=[[G, F]],
                base=0,
                channel_multiplier=F * G,
            )
            nc.vector.tensor_tensor(
                out=offs[:],
                in0=offs[:],
                in1=lab_raw[:].rearrange("p (f c) -> p f c", c=2)[:, :, 0],
                op=mybir.AluOpType.add,
            )

            vals = pool.tile([P, F], f32)
            for c in range(gather_chunks):
                sl = slice(c * fc, (c + 1) * fc)
                nc.gpsimd.indirect_dma_start(
                    out=vals[:, sl],
                    out_offset=None,
                    in_=x_t.ap(),
                    in_offset=bass.IndirectOffsetOnAxis(ap=offs[:, sl], axis=1),
                )

            # min(vals - margin, 0) == -relu(margin - vals); negated on host
            clamp_t = pool.tile([P, F], f32)
            nc.vector.tensor_scalar(
                out=clamp_t[:],
                in0=vals[:],
                scalar1=POSITIVE_MARGIN,
                scalar2=0.0,
                op0=mybir.AluOpType.subtract,
                op1=mybir.AluOpType.min,
            )
            acc = pool.tile([P, 1], f32)
            nc.vector.reduce_sum(acc[:], clamp_t[:], axis=mybir.AxisListType.X)
            nc.scalar.dma_start(out=out_t.ap(), in_=acc[:])

    nc.compile()
    return nc


_PROG = None


def _get_prog():
    global _PROG
    if _PROG is None:
        _PROG = build_program()
    return _PROG


def _make_in_maps(inputs: np.ndarray, labels: np.ndarray):
    inputs = np.asarray(inputs)
    labels = np.asarray(labels)
    assert inputs.shape == (N, G), inputs.shape
    assert labels.shape == (N,), labels.shape
    inputs = np.ascontiguousarray(inputs, dtype=np.float32)

    if labels.dtype == np.int64:
        lab2 = np.ascontiguousarray(labels).view(np.int32).reshape(N, 2)
    else:
        lab2 = np.zeros((N, 2), dtype=np.int32)
        lab2[:, 0] = labels.astype(np.int32)
    lab2 = np.ascontiguousarray(lab2)

    in_maps = []
    for c in range(NCORES):
        sl = slice(c * NS, (c + 1) * NS)
        in_maps.append({"inputs": inputs[sl], "labels_lo_hi": lab2[sl]})
    return in_maps


def _run(inputs, labels, trace: bool = False):
    nc = _get_prog()
    in_maps = _make_in_maps(inputs, labels)
    res = bass_utils.run_bass_kernel_spmd(
        nc, in_maps, core_ids=list(range(NCORES)), trace=trace
    )
    total = 0.0
    for r in res.results:
        total += float(np.asarray(r["partials"], dtype=np.float64).sum())
    out = np.array(-total / N, dtype=np.float32)
    return out, res


def kernel(inputs, labels):
    out, _ = _run(inputs, labels, trace=False)
    return out
